# revision 17
# baseline (speedup 1.0000x reference)
"""AdaptiveEdgeSmoothing Trainium2 kernel (v3).

Reference semantics (per sample, 1024x1024 f32 image):
    edges     = |conv3x3(mask, LAPLACIAN)|          (SAME zero pad)
    edge_mask = edges > 0.5*edge_sensitivity
    sm        = mask*(1-bf) + box5(mask)/25*bf,  bf = blur_strength/3
    result    = where(edge_mask, sm, mask)
    out       = (result > final_threshold).astype(f32)

Strategy: B=16 samples sharded 2-per-core across 8 NeuronCores (pure data
parallel).  Per core, 17 row-tiles: 8 tiles of 124 rows per image plus ONE
merged tile carrying both images' last 32 rows (img0 at partitions 0..34,
img1 at 64..98 — compute operands keep 32-partition quadrant alignment).
A tile's block holds rows s..s+126 at partitions 0..126 and the 2-row top
halo parked at partitions 126..128; the halos of all 7 mid tiles of an
image arrive in ONE strided SWDGE DMA.  Weights ride the same SWDGE queue
ahead of the first image blocks (HWDGE caps at ~4 DMA engines and made
the first matmul wait ~6us).

Per tile the TensorE computes, via column-shifted rhs views of the
zero-margined block and a precomputed u3 = x<<1 + x>>1 (split between
DVE and Pool by column halves):
    PSUM1 = 9x - box3(x)              (w3b@x + w3a@u3; the Laplacian)
    PSUM2 = x - sm = bf*x - bf/25*box5(x)
                                      (w5a@{x<<2,x>>2,u3} + w5b@x)
Mid tiles of an image share identical weights, so adjacent pairs are
emitted with interleaved matmuls: 4 LDWEIGHTS serve 2 tiles (PSUM holds
exactly 2x{p1,p2}).  The elementwise tail is 3 short ops:
    sq = PSUM1^2                               (ACT, psum->sbuf)
    L  = select(sq > thr^2, PSUM2, 0)          (EDGE_GATE_ANT, DVE)
    o  = (x - ft) > L  -> uint8                (SUB_GT_ANT, DVE, 2x mode)
which is exactly (result > ft): no edge -> x>ft; edge -> x-ft > x-sm <=>
sm>ft.  Output stores ride the otherwise idle sync-engine HWDGE queue.
"""

import sys

if '/opt/trn_rl_repo' not in sys.path:
    sys.path.insert(0, '/opt/trn_rl_repo')

import numpy as np

import concourse.bass as bass
import concourse.bacc as bacc
import concourse.bass_utils as bass_utils
import concourse.mybir as mybir
from concourse.tile import TileContext, add_dep_helper
from concourse.bass_utils import run_bass_kernel_spmd
from concourse import dve_ops as _dve_ops
from concourse.dve_spec import Spec, Src0, Src1, C0, Zero, select

# Enable walrus's LDWEIGHTS optimization for this kernel's compile:
# consecutive matmuls sharing a stationary operand skip redundant weight
# loads.  (The flag is hardcoded off in bir_verify_and_optimise.)
if not getattr(bass_utils, "_ldw_opt_patched", False):
    _orig_run_command = bass_utils.run_command

    def _run_command_ldw(argv, **kwargs):
        if isinstance(argv, list):
            argv = ["--enable-ldw-opt=true" if a == "--enable-ldw-opt=false"
                    else a for a in argv]
        return _orig_run_command(argv, **kwargs)

    bass_utils.run_command = _run_command_ldw
    bass_utils._ldw_opt_patched = True

# --- custom DVE ops ---------------------------------------------------------
EDGE_GATE = _dve_ops.DveOp(
    "EDGE_GATE_ANT",
    Spec(
        body=select(Src0 > C0, Src1, Zero),
        reference=lambda in0, in1, s0, s1, imm2: np.where(
            in0.astype(np.float32) > s0, in1, 0.0
        ).astype(np.float32),
    ),
    subdim=False,
    uops_sha={"v3": "e54edd49cbbf4900", "v4": "1a8a6c5fc1b3b863"},
)
USE_O2X = True
SUB_GT = _dve_ops.DveOp(
    "SUB_GT_ANT",
    Spec(
        body=(Src0 - C0) > Src1,
        reference=lambda in0, in1, s0, s1, imm2: (
            (in0.astype(np.float32) - s0) > in1
        ).astype(np.float32),
    ),
    subdim=False,
    uops_sha={"v3": "e0d402c4a448ef2d", "v4": "67c05032f428bc13"},
    perf_en={"v3": True, "v4": True},
)
for _op in (EDGE_GATE, SUB_GT):
    if _op.name not in _dve_ops._SUB_OPCODE_FOR_NAME:
        _dve_ops.OPS.append(_op)
        _dve_ops._SUB_OPCODE_FOR_NAME[_op.name] = (
            max(_dve_ops._SUB_OPCODE_FOR_NAME.values()) + 1
        )
        _dve_ops.CUSTOM_DVE_SPECS[_op.name] = _op.spec
        assert _dve_ops._SUB_OPCODE_FOR_NAME[_op.name] < 0x20

H = W = 1024
N_CORES = 8
IMGS_PER_CORE = 2
F32 = mybir.dt.float32
F32R = mybir.dt.float32r
U8 = mybir.dt.uint8
XP = 1028  # padded block pitch (2-col zero margins each side)

# tiles 0..16: (cls, img, s).  Weight classes:
#   cls 0/1: img0 t0 / img0 mid;  cls 2/3: img1 t0 / img1 mid;
#   cls 4: merged last-32-rows tile for both images.
TILES = []
for img in range(IMGS_PER_CORE):
    for t in range(8):
        TILES.append((img * 2 + (0 if t == 0 else 1), img, 124 * t))
TILES.append((4, 0, 992))  # merged: both images' rows 992..1024
N_TILES = len(TILES)  # 17

# emission units: mid-tile pairs share one LDWEIGHTS set per weight
UNITS = [[0], [1, 2], [3, 4], [5, 6], [7],
         [8], [9, 10], [11, 12], [13, 14], [15], [16]]

CLS_GEOM = {0: (126, 124), 1: (128, 124), 2: (126, 124), 3: (128, 124),
            4: (98, 96)}
CLS_KIND = {0: 0, 1: 1, 2: 0, 3: 1, 4: 2}


def _bands(kind):
    """Banded [128,128] masks b3 (|d|<=1), b5 (|d|<=2), ident (d==0) with
    d = row(k) - outrow(m) in tile-relative coords; absent rows clipped."""
    rows = np.full(128, 10 ** 6)
    outr = np.full(128, -10 ** 6)
    blk_k = np.zeros(128, np.int32)
    blk_m = np.zeros(128, np.int32)
    if kind in (0, 1):
        for k in range(126):
            rows[k] = k
        if kind == 1:
            rows[126] = -2
            rows[127] = -1
        for m in range(124):
            outr[m] = m
    else:
        for b in range(2):
            base = 64 * b
            for i in range(32):
                rows[base + i] = 2 + i      # rows 992..1024 (990-relative)
                outr[base + i] = 2 + i
                blk_k[base + i] = b
                blk_m[base + i] = b
            rows[base + 32] = 0             # row 990
            rows[base + 33] = 1             # row 991
            blk_k[base + 32] = blk_k[base + 33] = b
    b3 = np.zeros((128, 128), np.float32)
    b5 = np.zeros((128, 128), np.float32)
    idm = np.zeros((128, 128), np.float32)
    for m in range(128):
        if outr[m] < -1000:
            continue
        for k in range(128):
            if rows[k] > 1000 or blk_k[k] != blk_m[m]:
                continue
            d = rows[k] - outr[m]
            if abs(d) <= 1:
                b3[k, m] = 1.0
            if abs(d) <= 2:
                b5[k, m] = 1.0
            if d == 0:
                idm[k, m] = 1.0
    return b3, b5, idm


_BANDS = {kind: _bands(kind) for kind in range(3)}

_compiled = None
last_results = None


def _build():
    nc = bacc.Bacc("TRN2", target_bir_lowering=False, debug=False,
                   num_devices=N_CORES)
    x = nc.dram_tensor("x", [IMGS_PER_CORE, H, W], F32R,
                       kind="ExternalInput")
    wp = nc.dram_tensor("wp", [128, 5 * 4 * 128], F32R,
                        kind="ExternalInput").ap()
    thr2 = nc.dram_tensor("thr2", [IMGS_PER_CORE + 1, 128, 1], F32,
                          kind="ExternalInput").ap()
    ftd = nc.dram_tensor("ftd", [IMGS_PER_CORE + 1, 128, 1], F32,
                         kind="ExternalInput").ap()
    y = nc.dram_tensor("out", [IMGS_PER_CORE, H, W], U8,
                       kind="ExternalOutput")

    with TileContext(nc) as tc:
        with (
            tc.tile_pool(name="wpool", bufs=1) as wpool,
            tc.tile_pool(name="spool", bufs=1) as spool,
            tc.tile_pool(name="xpool", bufs=1) as xpool,
            tc.tile_pool(name="p1pool", bufs=2, space="PSUM") as p1pool,
            tc.tile_pool(name="p2pool", bufs=2, space="PSUM") as p2pool,
            tc.tile_pool(name="upool", bufs=4) as upool,
            tc.tile_pool(name="sqpool", bufs=3) as sqpool,
            tc.tile_pool(name="lpool", bufs=3) as lpool,
            tc.tile_pool(name="opool", bufs=4) as opool,
        ):
            # --- scalars via (idle) ACT HWDGE queue ----------------------
            sc_t = []
            for i in range(IMGS_PER_CORE + 1):
                t2 = spool.tile([128, 1], F32, tag=f"t2_{i}")
                f = spool.tile([128, 1], F32, tag=f"ft_{i}")
                nc.scalar.dma_start(out=t2[:], in_=thr2[i])
                nc.scalar.dma_start(out=f[:], in_=ftd[i])
                sc_t.append((t2, f))

            # --- x blocks + weights: SWDGE, ordered by first use ---------
            xbig = xpool.tile([128, N_TILES * XP], F32R, tag="xbig")
            x3 = xbig[:, :].rearrange("p (t c) -> p t c", c=XP)
            nc.vector.memset(x3[:, :, 0:2].bitcast(F32), 0)
            nc.vector.memset(x3[:, :, 1026:1028].bitcast(F32), 0)
            nc.gpsimd.memset(x3[32:64, 16, :].bitcast(F32), 0)

            wall = wpool.tile([128, 5 * 4 * 128], F32R, tag="wall")

            def w_ap(cls, j):
                b = (cls * 4 + j) * 128
                return wall[:, b:b + 128]

            def emit_load(j, gate=None):
                cls, img, s = TILES[j]
                if cls == 4:
                    ld = nc.gpsimd.dma_start(
                        out=x3[0:32, j, 2:1026],
                        in_=x.ap()[0, 992:1024, :])
                    nc.gpsimd.dma_start(
                        out=x3[32:34, j, 2:1026],
                        in_=x.ap()[0, 990:992, :])
                    nc.gpsimd.dma_start(
                        out=x3[64:96, j, 2:1026],
                        in_=x.ap()[1, 992:1024, :])
                    nc.gpsimd.dma_start(
                        out=x3[96:98, j, 2:1026],
                        in_=x.ap()[1, 990:992, :])
                else:
                    ld = nc.gpsimd.dma_start(
                        out=x3[0:126, j, 2:1026],
                        in_=x.ap()[img, s:s + 126, :])
                if gate is not None:
                    add_dep_helper(ld.ins, gate.ins, reason="stagger")

            def emit_halo(img, gate=None):
                j0 = img * 8 + 1
                ld = nc.gpsimd.dma_start(
                    out=x3[126:128, j0:j0 + 7, 2:1026],
                    in_=bass.AP(x, img * H * W + 122 * W,
                                [[W, 2], [124 * W, 7], [1, W]]))
                if gate is not None:
                    add_dep_helper(ld.ins, gate.ins, reason="stagger")

            # img0 weights -> first block -> img0 halo -> next blocks
            nc.gpsimd.dma_start(out=wall[:, 0:1024], in_=wp[:, 0:1024])
            emit_load(0)
            emit_halo(0)
            emit_load(1)
            emit_load(2)
            nc.gpsimd.dma_start(out=wall[:, 1024:2048], in_=wp[:, 1024:2048])
            nc.gpsimd.dma_start(out=wall[:, 2048:2560], in_=wp[:, 2048:2560])

            loaded = 3
            halo1_done = False

            # --- main loop over units ------------------------------------
            for ui, unit in enumerate(UNITS):
                tiles = [TILES[j] for j in unit]
                cls = tiles[0][0]
                k_tot, nout = CLS_GEOM[cls]
                u3s, p1s, p2s = {}, {}, {}
                for j in unit:
                    xt = x3[:, j, :]
                    u3 = upool.tile([128, 1024], F32R, tag="u3")
                    nc.gpsimd.tensor_tensor(
                        u3[0:k_tot, 512:1024],
                        xt[0:k_tot, 513:1025].bitcast(F32),
                        xt[0:k_tot, 515:1027].bitcast(F32),
                        mybir.AluOpType.add)
                    nc.vector.tensor_tensor(
                        u3[0:k_tot, 0:512],
                        xt[0:k_tot, 1:513].bitcast(F32),
                        xt[0:k_tot, 3:515].bitcast(F32),
                        mybir.AluOpType.add)
                    u3s[j] = u3
                    p1 = p1pool.tile([128, 1024], F32, tag="p1")
                    p2 = p2pool.tile([128, 1024], F32, tag="p2")
                    p1s[j] = p1
                    p2s[j] = p2

                # weight groups: (psum dict, wj, rhs shifts, start, stop)
                groups = [
                    (p1s, 1, (0,), True, False),
                    (p1s, 0, (None,), False, True),
                    (p2s, 2, (-2, 2, None), True, False),
                    (p2s, 3, (0,), False, True),
                ]
                first_mm = None
                for psd, wj, shifts, st, sp in groups:
                    for j in unit:
                        xt = x3[:, j, :]
                        for si, sh in enumerate(shifts):
                            for c in (0, 512):
                                if sh is None:
                                    rhs = u3s[j][0:k_tot, c:c + 512]
                                else:
                                    rhs = xt[0:k_tot,
                                             2 + sh + c:2 + sh + c + 512]
                                mm = nc.tensor.matmul(
                                    psd[j][0:nout, c:c + 512],
                                    w_ap(cls, wj)[0:k_tot, 0:nout],
                                    rhs, start=st and si == 0,
                                    stop=sp and si == len(shifts) - 1)
                                if first_mm is None:
                                    first_mm = mm

                for j in unit:
                    _, img, s = TILES[j]
                    sci = 2 if cls == 4 else img
                    t2_ap, ft_ap = sc_t[sci]
                    xt = x3[:, j, :]
                    sq_t = sqpool.tile([128, 1024], F32, tag="sq")
                    nc.scalar.activation(sq_t[0:nout, :], p1s[j][0:nout, :],
                                         mybir.ActivationFunctionType.Square)
                    lt = lpool.tile([128, 1024], F32, tag="L")
                    nc.vector._custom_dve(
                        EDGE_GATE, out=lt[0:nout, :], in0=sq_t[0:nout, :],
                        in1=p2s[j][0:nout, :], s0=t2_ap[0:nout, :])
                    o_t = opool.tile([128, 1024], U8, tag="o")
                    if USE_O2X:
                        nc.vector._custom_dve(
                            SUB_GT, out=o_t[0:nout, :],
                            in0=xt[0:nout, 2:1026].bitcast(F32),
                            in1=lt[0:nout, :], s0=ft_ap[0:nout, :])
                    else:
                        nc.vector.scalar_tensor_tensor(
                            o_t[0:nout, :],
                            xt[0:nout, 2:1026].bitcast(F32),
                            ft_ap[0:nout, :],
                            lt[0:nout, :],
                            mybir.AluOpType.subtract,
                            mybir.AluOpType.is_gt)
                    if cls == 4:
                        nc.sync.dma_start(out=y.ap()[0, 992:1024, :],
                                          in_=o_t[0:32, :])
                        nc.sync.dma_start(out=y.ap()[1, 992:1024, :],
                                          in_=o_t[64:96, :])
                    else:
                        nc.sync.dma_start(out=y.ap()[img, s:s + nout, :],
                                          in_=o_t[0:nout, :])

                # staggered prefetch: stay ~4 tiles ahead
                want = min(unit[-1] + 4, N_TILES - 1)
                while loaded <= want:
                    emit_load(loaded, first_mm)
                    loaded += 1
                if not halo1_done and unit[-1] >= 6:
                    emit_halo(1, first_mm)
                    halo1_done = True
    nc.compile()
    return nc


def _in_maps(mask, blur_strength, edge_sensitivity, final_threshold):
    mask = np.ascontiguousarray(mask.reshape(16, H, W), np.float32)
    bs = np.asarray(blur_strength, np.float32).reshape(16)
    es = np.asarray(edge_sensitivity, np.float32).reshape(16)
    fts = np.asarray(final_threshold, np.float32).reshape(16)

    maps = []
    for c in range(N_CORES):
        ii = [2 * c, 2 * c + 1]
        bf = [float(bs[i]) / 3.0 for i in ii]
        wp = np.zeros((5, 4, 128, 128), np.float32)
        for cls in range(5):
            kind = CLS_KIND[cls]
            b3, b5, idm = _BANDS[kind]
            wp[cls, 0] = -b3
            wp[cls, 1] = 9.0 * idm - b3
            if cls == 4:
                for bi, k0 in enumerate((0, 64)):
                    k1 = k0 + 34
                    wp[cls, 2][k0:k1] = -(bf[bi] / 25.0) * b5[k0:k1]
                    wp[cls, 3][k0:k1] = (bf[bi] * idm[k0:k1]
                                         - (bf[bi] / 25.0) * b5[k0:k1])
            else:
                b = bf[cls // 2]
                wp[cls, 2] = -(b / 25.0) * b5
                wp[cls, 3] = b * idm - (b / 25.0) * b5
        wpf = np.ascontiguousarray(
            wp.transpose(2, 0, 1, 3).reshape(128, 5 * 4 * 128))

        t2m = np.zeros((IMGS_PER_CORE + 1, 128, 1), np.float32)
        ftm = np.zeros((IMGS_PER_CORE + 1, 128, 1), np.float32)
        for i in range(IMGS_PER_CORE):
            t2m[i, :, 0] = (0.5 * es[ii[i]]) ** 2
            ftm[i, :, 0] = fts[ii[i]]
        t2m[2, 0:64, 0] = (0.5 * es[ii[0]]) ** 2
        t2m[2, 64:128, 0] = (0.5 * es[ii[1]]) ** 2
        ftm[2, 0:64, 0] = fts[ii[0]]
        ftm[2, 64:128, 0] = fts[ii[1]]

        maps.append({
            "x": np.ascontiguousarray(mask[ii]),
            "wp": wpf,
            "thr2": t2m,
            "ftd": ftm,
        })
    return maps


def kernel(mask, blur_strength, edge_sensitivity, final_threshold):
    global _compiled, last_results
    if _compiled is None:
        _compiled = _build()
    maps = _in_maps(mask, blur_strength, edge_sensitivity, final_threshold)
    res = run_bass_kernel_spmd(_compiled, maps, core_ids=list(range(N_CORES)))
    last_results = res
    out = np.empty((16, 1, H, W), np.float32)
    for c in range(N_CORES):
        out[2 * c:2 * c + 2, 0] = res.results[c]["out"]
    return out


# revision 18
# speedup vs baseline: 1.0985x; 1.0985x over previous
"""AdaptiveEdgeSmoothing Trainium2 kernel.

Reference semantics (per sample, 1024x1024 f32 image):
    edges     = |conv3x3(mask, LAPLACIAN)|          (SAME zero pad)
    edge_mask = edges > 0.5*edge_sensitivity
    sm        = mask*(1-bf) + box5(mask)/25*bf,  bf = blur_strength/3
    result    = where(edge_mask, sm, mask)
    out       = (result > final_threshold).astype(f32)

Strategy: B=16 samples sharded 2-per-core across 8 NeuronCores (pure data
parallel).  Per core, each image is processed in 9 row-tiles (rows on
partitions, cols on the free axis).  All convolution arithmetic runs on the
TensorEngine as banded fp32r matmuls over column-shifted rhs views of
zero-margined SBUF blocks:
    PSUM1 = 9x - box3(x)            (3 accumulating passes; the Laplacian)
    PSUM2 = (bf/25)*box5(x)+(1-bf)x (5 passes; the smoothed value)
Vertical band weights (incl. SAME-pad clipping and the per-sample bf
scaling) are precomputed in numpy and DMA'd in.  Halo rows are parked at
spare partitions so output rows start at partition 0 on every operand.
Row-tiles are packed side by side in the free axis of big per-image SBUF
buffers so that loads and stores are a few >1MiB SWDGE (gpsimd) DMAs,
which spread across all 16 SDMA engines (HWDGE transfers chunk
32-partitions-per-engine and cap at ~4 engines).  Elementwise tail: ACT
computes Relu(|lap| - thr) as an edge mask (nonzero = edge), DVE
copy_predicated overwrites a copy of x with sm where masked, then one
is_gt against final_threshold writes the packed output block.
"""

import sys

if '/opt/trn_rl_repo' not in sys.path:
    sys.path.insert(0, '/opt/trn_rl_repo')

import numpy as np

import concourse.bass as bass
import concourse.bacc as bacc
import concourse.bass_utils as bass_utils
import concourse.mybir as mybir
from concourse.tile import TileContext, add_dep_helper
from concourse.bass_utils import run_bass_kernel_spmd

# Enable walrus's LDWEIGHTS optimization for this kernel's compile:
# consecutive matmuls sharing a stationary operand skip redundant weight
# loads.  (The flag is hardcoded off in bir_verify_and_optimise.)
if not getattr(bass_utils, "_ldw_opt_patched", False):
    _orig_run_command = bass_utils.run_command

    def _run_command_ldw(argv, **kwargs):
        if isinstance(argv, list):
            argv = ["--enable-ldw-opt=true" if a == "--enable-ldw-opt=false"
                    else a for a in argv]
        return _orig_run_command(argv, **kwargs)

    bass_utils.run_command = _run_command_ldw
    bass_utils._ldw_opt_patched = True

H = W = 1024
N_CORES = 8
IMGS_PER_CORE = 2
F32 = mybir.dt.float32
F32R = mybir.dt.float32r
XP = 1028  # padded block pitch (2-col zero margins each side)

# tile geometry: (out_row_start, n_out, K_data, halo_partition_base, var)
# partitions [0, K_data) hold rows [s, s+K_data); partitions
# [halo_base, halo_base+2) hold rows [s-2, s).
TILES = [(124 * t, 124, 126, 126, (0 if t == 0 else 1)) for t in range(8)]
TILES.append((992, 32, 32, 32, 2))


def _band_templates():
    """Per variant: (V3, V5, I) as [128,128] f32, plus (K_total, nout)."""
    out = []
    for var in range(3):
        s, nout, kd, hb, _ = TILES[0 if var == 0 else (1 if var == 1 else 8)]
        v3 = np.zeros((128, 128), np.float32)
        v5 = np.zeros((128, 128), np.float32)
        ident = np.zeros((128, 128), np.float32)
        for k in range(kd):
            for p in range(nout):
                d = k - p
                if abs(d) <= 1:
                    v3[k, p] = 1.0
                if abs(d) <= 2:
                    v5[k, p] = 1.0
                if d == 0:
                    ident[k, p] = 1.0
        if var != 0:  # top halo rows: partition hb+j holds row s-2+j
            for j in range(2):
                for p in range(nout):
                    d = (j - 2) - p
                    if abs(d) <= 1:
                        v3[hb + j, p] = 1.0
                    if abs(d) <= 2:
                        v5[hb + j, p] = 1.0
        k_tot = 128 if var != 2 else 34
        out.append((v3, v5, ident, k_tot, nout))
    return out


_TEMPLATES = _band_templates()

_compiled = None
last_results = None


def _margin_memsets(nc, blk, nblocks):
    """Zero the 2-col margins of every 1028-wide block in `blk`."""
    nc.vector.memset(blk[:, 0:2].bitcast(F32), 0)
    if nblocks > 1:
        # right margin of block t + left margin of block t+1 are contiguous
        spans = blk[:, 1026:1026 + (nblocks - 1) * XP].rearrange(
            "p (t c) -> p t c", c=XP)[:, :, 0:4]
        nc.vector.memset(spans.bitcast(F32), 0)
    nc.vector.memset(
        blk[:, nblocks * XP - 2:nblocks * XP].bitcast(F32), 0)


def _build():
    nc = bacc.Bacc("TRN2", target_bir_lowering=False, debug=False,
                   num_devices=N_CORES)
    x = nc.dram_tensor("x", [IMGS_PER_CORE, H, W], F32R,
                       kind="ExternalInput")
    w3p = nc.dram_tensor("w3p", [128, 3 * 2 * 128], F32R,
                         kind="ExternalInput").ap()
    w5p = nc.dram_tensor("w5p", [128, IMGS_PER_CORE * 3 * 2 * 128], F32R,
                         kind="ExternalInput").ap()
    negthr = nc.dram_tensor("negthr", [IMGS_PER_CORE, 128, 1], F32,
                            kind="ExternalInput").ap()
    ft = nc.dram_tensor("ft", [IMGS_PER_CORE, 128, 1], F32,
                        kind="ExternalInput").ap()
    y = nc.dram_tensor("out", [IMGS_PER_CORE, H, W], mybir.dt.uint8,
                       kind="ExternalOutput")

    def xdma(img, out_ap, row0, nrows, ntiles):
        """DRAM read AP: partition p, block t -> image row row0 + 124t + p."""
        return nc.gpsimd.dma_start(
            out=out_ap,
            in_=bass.AP(x, img * H * W + row0 * W,
                        [[W, nrows], [124 * W, ntiles], [1, W]]))

    with TileContext(nc) as tc:
        with (
            tc.tile_pool(name="wpool", bufs=1) as wpool,
            tc.tile_pool(name="spool", bufs=1) as spool,
            tc.tile_pool(name="xpool", bufs=1) as xpool,
            tc.tile_pool(name="p1pool", bufs=2, space="PSUM") as p1pool,
            tc.tile_pool(name="p2pool", bufs=2, space="PSUM") as p2pool,
            tc.tile_pool(name="apool", bufs=3) as apool,
            tc.tile_pool(name="empool", bufs=3) as empool,
            tc.tile_pool(name="vpool", bufs=3) as vpool,
            tc.tile_pool(name="upool", bufs=4) as upool,
            tc.tile_pool(name="opool", bufs=6) as opool,
        ):
            # --- one-time loads: weights + per-image scalars -------------
            # (HWDGE so they don't queue ahead of image data on the SWDGE
            # rings)
            w3all = wpool.tile([128, 3 * 2 * 128], F32R, tag="w3all")
            nc.scalar.dma_start(out=w3all[:], in_=w3p)
            w5all = wpool.tile([128, IMGS_PER_CORE * 3 * 2 * 128], F32R,
                               tag="w5all")
            # per-image halves so image 0's weights land first
            nc.scalar.dma_start(out=w5all[:, 0:768], in_=w5p[:, 0:768])
            nc.scalar.dma_start(out=w5all[:, 768:1536], in_=w5p[:, 768:1536])

            def w3_ap(v, sc):
                return w3all[:, (v * 2 + sc) * 128:(v * 2 + sc) * 128 + 128]

            def w5_ap(img, v, sc):
                base = ((img * 3 + v) * 2 + sc) * 128
                return w5all[:, base:base + 128]

            sc_t = []
            for img in range(IMGS_PER_CORE):
                nt = spool.tile([128, 1], F32, tag=f"nt{img}")
                f = spool.tile([128, 1], F32, tag=f"ft{img}")
                nc.scalar.dma_start(out=nt[:], in_=negthr[img])
                nc.scalar.dma_start(out=f[:], in_=ft[img])
                sc_t.append((nt, f))

            def emit_group(img, k, gate=None):
                """Load 2-block group k (tiles 2k, 2k+1); k=4 is tile 8."""
                if k < 4:
                    g = xpool.tile([128, 2 * XP], F32R, tag=f"x{img}g{k}")
                    _margin_memsets(nc, g, 2)
                    g3 = g[:, :].rearrange("p (t c) -> p t c", c=XP)
                    s0 = 124 * 2 * k
                    # per-block main loads: finer completion granularity so
                    # the first tile's matmuls start sooner
                    if img == 0 and k == 0:
                        # very first block via sync HWDGE: issues earliest
                        nc.sync.dma_start(out=g3[0:126, 0, 2:1026],
                                          in_=x.ap()[0, 0:126, :])
                        nc.sync.dma_start(out=g3[126:128, 0, 2:1026],
                                          in_=x.ap()[0, 0:2, :])
                    else:
                        ld = xdma(img, g3[0:126, 0:1, 2:1026], s0, 126, 1)
                        if gate is not None:
                            add_dep_helper(ld.ins, gate.ins,
                                           reason="stagger")
                        if k == 0:  # t0 halo rows are zero-weight dummies
                            xdma(img, g3[126:128, 0:1, 2:1026], 0, 2, 1)
                        else:
                            xdma(img, g3[126:128, 0:1, 2:1026], s0 - 2, 2, 1)
                    xdma(img, g3[0:126, 1:2, 2:1026], s0 + 124, 126, 1)
                    xdma(img, g3[126:128, 1:2, 2:1026], s0 + 122, 2, 1)
                else:
                    g = xpool.tile([128, XP], F32R, tag=f"x{img}g4")
                    _margin_memsets(nc, g, 1)
                    g3 = g[:, :].rearrange("p (t c) -> p t c", c=XP)
                    ld = nc.gpsimd.dma_start(out=g3[0:32, 0, 2:1026],
                                             in_=x.ap()[img, 992:1024, :])
                    if gate is not None:
                        add_dep_helper(ld.ins, gate.ins, reason="stagger")
                    nc.gpsimd.dma_start(out=g3[32:34, 0, 2:1026],
                                        in_=x.ap()[img, 990:992, :])
                return g3

            # --- main loop ----------------------------------------------
            SEQ = list(range(9))
            xg = {(0, 0): emit_group(0, 0)}
            first_mm = None
            tile_mm = {}
            # emitted after tile (img,t): list of (img, group, gate_tile)
            prefetch = {(0, 0): [(0, 1, (0, 0)), (0, 2, (0, 0))],
                        (0, 1): [(0, 3, (0, 1))],
                        (0, 2): [(0, 4, (0, 2))],
                        (0, 3): [(1, 0, (0, 3))],
                        (0, 5): [(1, 1, (0, 5))],
                        (0, 7): [(1, 2, (0, 7))],
                        (1, 0): [(1, 3, (1, 0))],
                        (1, 2): [(1, 4, (1, 2))]}
            for img in range(IMGS_PER_CORE):
                nt_ap, ft_ap = sc_t[img]
                for t in SEQ:
                    s, nout, kd, hb, var = TILES[t]
                    k_tot = _TEMPLATES[var][3]
                    xt3 = xg[(img, min(t // 2, 4))]
                    blk = t % 2 if t < 8 else 0

                    p1 = p1pool.tile([128, 1024], F32, tag="p1")
                    p2 = p2pool.tile([128, 1024], F32, tag="p2")
                    # On alternating tiles, compute the +-2 horizontal
                    # taps as one DVE add (u = x<<2 + x>>2), replacing two
                    # PE passes with one pass over u (PE/DVE balance).
                    # high_priority orders the add ahead of the psum-gated
                    # DVE tail ops so the PE is not stalled.
                    use_u = first_mm is not None
                    u_t = None
                    if use_u:
                        u_t = upool.tile([128, 1024], F32R, tag="u")
                        with tc.high_priority(offset=60):
                            nc.vector.tensor_tensor(
                                u_t[0:k_tot, :],
                                xt3[0:k_tot, blk, 0:1024].bitcast(F32),
                                xt3[0:k_tot, blk, 4:1028].bitcast(F32),
                                mybir.AluOpType.add)
                        w5shifts = (-1, 1, None)
                    else:
                        w5shifts = (-2, -1, 1, 2)
                    groups = [
                        (p1, w3_ap(var, 0), (-1, 1), False),
                        (p1, w3_ap(var, 1), (0,), True),
                        (p2, w5_ap(img, var, 0), w5shifts, False),
                        (p2, w5_ap(img, var, 1), (0,), True),
                    ]
                    for ps, wt, shifts, is_last in groups:
                        first = shifts[0] in (-1, -2)
                        for si, sh in enumerate(shifts):
                            for c in (0, 512):
                                if sh is None:
                                    rhs = u_t[0:k_tot, c:c + 512]
                                else:
                                    rhs = xt3[0:k_tot, blk,
                                              2 + sh + c:2 + sh + c + 512]
                                mm = nc.tensor.matmul(
                                    ps[0:nout, c:c + 512],
                                    wt[0:k_tot, 0:nout],
                                    rhs,
                                    start=(first and si == 0),
                                    stop=is_last)
                                if first_mm is None:
                                    first_mm = mm
                                tile_mm.setdefault((img, t), mm)

                    # edge mask: nonzero where |lap| > thr
                    a_t = apool.tile([128, 1024], F32, tag="a")
                    em_t = empool.tile([128, 1024], F32, tag="em")
                    nc.scalar.activation(a_t[0:nout, :], p1[0:nout, :],
                                         mybir.ActivationFunctionType.Abs)
                    nc.scalar.activation(em_t[0:nout, :], a_t[0:nout, :],
                                         mybir.ActivationFunctionType.Relu,
                                         bias=nt_ap[0:nout, :])
                    # v = x; v <- sm where edge; out-block = (v > ft)
                    v_t = vpool.tile([128, 1024], F32, tag="v")
                    nc.scalar.copy(v_t[0:nout, :],
                                   xt3[0:nout, blk, 2:1026].bitcast(F32))
                    nc.vector.copy_predicated(v_t[0:nout, :],
                                              em_t[0:nout, :]
                                              .bitcast(mybir.dt.int32),
                                              p2[0:nout, 0:1024])
                    o_t = opool.tile([128, 1024], mybir.dt.uint8, tag="o")
                    nc.vector.tensor_scalar(o_t[0:nout, :],
                                            v_t[0:nout, :],
                                            ft_ap[0:nout, :], None,
                                            mybir.AluOpType.is_gt)
                    nc.gpsimd.dma_start(out=y.ap()[img, s:s + nout, :],
                                        in_=o_t[0:nout, :])

                    # staggered prefetch: each load group starts only after
                    # an earlier tile's compute has begun, so its packets
                    # don't steal SDMA slots from data needed sooner
                    for job in prefetch.get((img, t), []):
                        jimg, jk, jgate = job
                        xg[(jimg, jk)] = emit_group(jimg, jk,
                                                    tile_mm[jgate])
    nc.compile()
    return nc


def _in_maps(mask, blur_strength, edge_sensitivity, final_threshold):
    mask = np.ascontiguousarray(mask.reshape(16, H, W), np.float32)
    bs = np.asarray(blur_strength, np.float32).reshape(16)
    es = np.asarray(edge_sensitivity, np.float32).reshape(16)
    fts = np.asarray(final_threshold, np.float32).reshape(16)

    w3 = np.zeros((3, 2, 128, 128), np.float32)
    for v, (v3, v5t, ident, k_tot, nout) in enumerate(_TEMPLATES):
        w3[v, 0] = -v3
        w3[v, 1] = 9.0 * ident - v3
    w3p = np.ascontiguousarray(
        w3.transpose(2, 0, 1, 3).reshape(128, 3 * 2 * 128))

    maps = []
    for c in range(N_CORES):
        sel = slice(2 * c, 2 * c + 2)
        w5 = np.zeros((IMGS_PER_CORE, 3, 2, 128, 128), np.float32)
        for i in range(IMGS_PER_CORE):
            bf = bs[2 * c + i] / 3.0
            for v, (v3, v5t, ident, k_tot, nout) in enumerate(_TEMPLATES):
                w5[i, v, 0] = (bf / 25.0) * v5t
                w5[i, v, 1] = (bf / 25.0) * v5t + (1.0 - bf) * ident
        w5p = np.ascontiguousarray(
            w5.transpose(3, 0, 1, 2, 4).reshape(
                128, IMGS_PER_CORE * 3 * 2 * 128))
        negthr = np.zeros((IMGS_PER_CORE, 128, 1), np.float32)
        ftm = np.zeros((IMGS_PER_CORE, 128, 1), np.float32)
        for i in range(IMGS_PER_CORE):
            negthr[i, :, 0] = -(0.5 * es[2 * c + i])
            ftm[i, :, 0] = fts[2 * c + i]
        maps.append({
            "x": np.ascontiguousarray(mask[sel]),
            "w3p": w3p,
            "w5p": w5p,
            "negthr": negthr,
            "ft": ftm,
        })
    return maps


def kernel(mask, blur_strength, edge_sensitivity, final_threshold):
    global _compiled, last_results
    if _compiled is None:
        _compiled = _build()
    maps = _in_maps(mask, blur_strength, edge_sensitivity, final_threshold)
    res = run_bass_kernel_spmd(_compiled, maps, core_ids=list(range(N_CORES)))
    last_results = res
    out = np.empty((16, 1, H, W), np.float32)
    for c in range(N_CORES):
        out[2 * c:2 * c + 2, 0] = res.results[c]["out"]  # u8 {0,1} -> f32
    return out
